# revision 6
# baseline (speedup 1.0000x reference)
"""Trainium2 Bass kernel for nn_DecoderBlock (dense transformer decoder block).

Sharding: 8 cores = 4 batches x 2 query-sets of 512. Queries are interleaved
in 64-blocks (core h takes blocks {2j+h}): every query block j then needs
exactly j+1 key blocks, so both cores run the IDENTICAL causal program
(score/exp/z matmuls shrink toward the causal roofline) with only the data
(query packing + one [128,64] diagonal mask) differing per core.

On-chip layout: activations are feature-major ([feature partitions, seq
free]) so matmuls chain with no transposes; the host transposes at the
boundary. Big GEMMs run in bf16 (halves DMA + enables fast weight load);
PSUM accumulation, softmax denominators and LN statistics stay f32.
Softmax runs without max-subtraction (logits are bounded); the denominator
comes from a ones-column appended to V. Per-head normalization is deferred:
raw z and denominators are banked per head, then one batched [16,512]
reciprocal + one selector-matmul broadcast per feature tile applies it —
nothing slow sits between the PE and the next head's matmuls.
"""
import numpy as np

import concourse.bacc as bacc
import concourse.mybir as mybir
from concourse import tile

D = 1024
H = 16
DK = 64
FFN = 4096
B = 4
S = 1024
SQ = 512          # queries per core
NT = D // 128     # feature tiles
NF = FFN // 128
EPS = 1e-5
MASK_NEG = -6.0e4

F32 = mybir.dt.float32
F32R = mybir.dt.float32r
FP16 = mybir.dt.float16
AF = mybir.ActivationFunctionType
OP = mybir.AluOpType

# column indices in the packed per-feature table `cols`
C_G1, C_BE1, C_G2, C_BE2, C_G3, C_BE3, C_BQS, C_BKS, C_BQX, C_BKX, \
    C_BO, C_B2 = range(12)


def build_decoder(loop_k=1):
    nc = bacc.Bacc("TRN2", target_bir_lowering=False, debug=False,
                   num_devices=8)
    dp = nc.declare_dram_parameter
    xT_d = dp("xT", [D, S], FP16, isOutput=False)
    xqT_d = dp("xqT", [D, SQ], FP16, isOutput=False)
    encT_d = dp("encT", [D, S], FP16, isOutput=False)
    # pre-tiled weight slabs (host-prepared, contiguous per slab):
    # wq/wk: [8 slabs, 128, 1024]  slab t = W[:, 128t:128(t+1)] as [p, d*128+c]
    # wv: [16 slabs, 128, 512]     slab s*8+d = W[128d:128(d+1), 512s:512(s+1)]
    wq_s_d = dp("wq_s", [NT, 128, NT * 128], FP16, isOutput=False)
    wk_s_d = dp("wk_s", [NT, 128, NT * 128], FP16, isOutput=False)
    wv_s_d = dp("wv_s", [2 * NT, 128, 512], FP16, isOutput=False)
    wq_x_d = dp("wq_x", [NT, 128, NT * 128], FP16, isOutput=False)
    wk_x_d = dp("wk_x", [NT, 128, NT * 128], FP16, isOutput=False)
    wv_x_d = dp("wv_x", [2 * NT, 128, 512], FP16, isOutput=False)
    bv_s_d = dp("bv_s", [1, D], F32R, isOutput=False)
    bv_x_d = dp("bv_x", [1, D], F32R, isOutput=False)
    wo_d = dp("wo", [NT, 128, NT * 128], FP16, isOutput=False)
    w1_d = dp("w1", [NF, 128, NT * 128], FP16, isOutput=False)
    b1c_d = dp("b1c", [128, NF], F32, isOutput=False)
    w2_d = dp("w2", [NT, 128, NF * 128], FP16, isOutput=False)
    cols_d = dp("cols", [D, 12], F32, isOutput=False)
    onesr_d = dp("onesr", [1, SQ], F32R, isOutput=False)
    onescol_d = dp("onescol", [128, 1], FP16, isOutput=False)
    vones_d = dp("vones", [128, H], FP16, isOutput=False)
    # head-pair selector for the denominator broadcast:
    # sel[:, 128t:128(t+1)].T @ rcp -> rows 0:64 = rcp[2t], 64:128 = rcp[2t+1]
    sel_d = dp("sel", [H, NT * 128], F32R, isOutput=False)
    maskq_d = dp("maskq", [128, 64], FP16, isOutput=False)
    out_d = dp("out", [D, SQ], F32, isOutput=True)

    with tile.TileContext(nc) as tc, \
         nc.allow_low_precision(reason="bf16 gemms are intentional"), \
         tc.tile_pool(name="pers", bufs=1) as pers:
        def body(_iv=None):
            # ------------------ persistent small tensors --------------------
            # (emitted in DMA-priority order: v-phase inputs first)
            onesr = pers.tile([1, SQ], F32R, tag="onesr", name="onesr")
            onescol = pers.tile([128, 1], FP16, tag="onescol", name="onescol")
            vones = pers.tile([128, H], FP16, tag="vones", name="vones")
            nc.sync.dma_start(onesr[:], onesr_d[:, :])
            nc.sync.dma_start(onescol[:], onescol_d[:, :])
            nc.sync.dma_start(vones[:], vones_d[:, :])
            cols = [pers.tile([128, 12], F32, tag=f"cols{t}",
                              name=f"cols{t}") for t in range(NT)]
            sel = pers.tile([H, NT * 128], F32R, tag="sel", name="sel")

            x1T = [pers.tile([128, SQ], FP16, tag=f"x1T{t}",
                             name=f"x1T{t}") for t in range(NT)]
            x2T = [pers.tile([128, SQ], FP16, tag=f"x2T{t}",
                             name=f"x2T{t}") for t in range(NT)]

            def load_small_tensors():
                for t in range(NT):
                    nc.sync.dma_start(cols[t][:],
                                      cols_d[128*t:128*(t+1), :])
                nc.sync.dma_start(sel[:], sel_d[:, :])

            # ------------------ attention building block --------------------
            def attention_v(attp, srcT, wv_dram, bv_dram):
                """V projection (seq-major) + ones column -> vaS tiles."""
                nsk = S // 128
                bv = attp.tile([1, D], F32R, tag="bv", name="bv")
                nc.sync.dma_start(bv[:], bv_dram[:, :])
                vaS = [attp.tile([128, H * 65], FP16, tag=f"vaS{i}",
                                 name=f"va{i}") for i in range(nsk)]
                for i in range(nsk):
                    nc.vector.tensor_copy(
                        vaS[i][:, :].rearrange(
                            "p (h c) -> p h c", c=65)[:, :, 64:65],
                        vones[:, :].rearrange("p (h c) -> p h c", c=1))
                with tc.tile_pool(name="vw", bufs=1) as vw, \
                     tc.tile_pool(name="vps", bufs=2, space="PSUM") as vps:
                    for s in range(2):
                        wvs = []
                        for d in range(NT):
                            wv = vw.tile([128, 512], FP16, tag=f"wv{d}",
                                         name=f"wv{s}_{d}")
                            nc.sync.dma_start(wv[:],
                                              wv_dram[NT*s + d, :, :])
                            wvs.append(wv)
                        for i in range(nsk):
                            ps = vps.tile([128, 512], F32, tag="vp",
                                          name=f"vp{s}_{i}")
                            for d in range(NT):
                                nc.tensor.matmul(
                                    ps[:], srcT[d][:, 128*i:128*(i+1)],
                                    wvs[d][:], start=(d == 0), stop=False)
                            nc.tensor.matmul(
                                ps[:], onesr[0:1, 0:128],
                                bv[0:1, 512*s:512*(s+1)],
                                start=False, stop=True)
                            nc.scalar.activation(
                                vaS[i][:, 65*8*s: 65*8*(s+1)].rearrange(
                                    "p (h c) -> p h c", c=65)[:, :, 0:64],
                                ps[:, :].rearrange(
                                    "p (h c) -> p h c", c=64),
                                AF.Copy)
                return vaS

            def attention(attp, zTp, srcT, wq_dram, wk_dram, vaS,
                          cq, ck, q_from, maskq, den):
                """Q/K projections + scores/softmax/z per head pair.

                Raw (unnormalized) z goes to zT tiles; per-head softmax
                denominators go to den[16, SQ] (staged on one partition —
                engines can't write partition base hh — then scattered by
                one SBUF->SBUF DMA). causal iff maskq given."""
                nsk = S // 128
                zT = [zTp.tile([128, SQ], FP16, tag=f"zT{t}",
                               name=f"zT{t}") for t in range(NT)]
                stg = attp.tile([1, H * SQ], F32, tag="denstg",
                                name="denstg")
                with tc.tile_pool(name="qkw", bufs=3) as qkw, \
                     tc.tile_pool(name="qkloc", bufs=1) as qkloc, \
                     tc.tile_pool(name="qkps", bufs=1, space="PSUM") as qkps, \
                     tc.tile_pool(name="scps", bufs=3, space="PSUM") as scps, \
                     tc.tile_pool(name="zps", bufs=2, space="PSUM") as zps, \
                     tc.tile_pool(name="sexp", bufs=3) as sexp:
                    for t in range(NT):
                        qslab = qkw.tile([128, NT * 128], FP16, tag="qkslab",
                                         name=f"qslab{t}")
                        nc.sync.dma_start(qslab[:, :], wq_dram[t, :, :])
                        qps = qkps.tile([128, SQ], F32, tag="qps",
                                        name=f"qps{t}")
                        for d in range(NT):
                            nc.tensor.matmul(qps[:],
                                             qslab[:, 128*d:128*(d+1)],
                                             q_from[d][:, :],
                                             start=(d == 0),
                                             stop=(d == NT - 1))
                        qT = qkloc.tile([128, SQ], FP16, tag="qT",
                                        name=f"qT{t}", bufs=2)
                        nc.vector.tensor_scalar_add(qT[:], qps[:],
                                                    cols[t][:, cq:cq+1])
                        kslab = qkw.tile([128, NT * 128], FP16, tag="qkslab",
                                         name=f"kslab{t}")
                        nc.sync.dma_start(kslab[:, :], wk_dram[t, :, :])
                        kT = qkloc.tile([128, S], FP16, tag="kT",
                                        name=f"kT{t}", bufs=2)
                        for s in range(2):
                            kps = qkps.tile([128, 512], F32, tag="kps",
                                            name=f"kps{t}_{s}", bufs=2)
                            for d in range(NT):
                                nc.tensor.matmul(
                                    kps[:], kslab[:, 128*d:128*(d+1)],
                                    srcT[d][:, 512*s:512*(s+1)],
                                    start=(d == 0), stop=(d == NT - 1))
                            nc.vector.tensor_scalar_add(
                                kT[:, 512*s:512*(s+1)], kps[:],
                                cols[t][:, ck:ck+1])
                        for hh in (2*t, 2*t + 1):
                            lo = 64 * (hh % 2)
                            zp = zps.tile([65, SQ], F32, tag="zp",
                                          name=f"zp{hh}")
                            for i in range(nsk):
                                off = 64 * i if maskq is not None else 0
                                ncol = SQ - off
                                scw = scps.tile([128, SQ], F32, tag="scw",
                                                name=f"scw{hh}_{i}")
                                nc.tensor.matmul(
                                    scw[:, 0:ncol],
                                    kT[lo:lo+64, 128*i:128*(i+1)],
                                    qT[lo:lo+64, off:SQ],
                                    start=True, stop=True)
                                if maskq is not None:
                                    # diagonal block: queries 64i..64i+63
                                    nc.vector.tensor_tensor(
                                        scw[:, 0:64], scw[:, 0:64],
                                        maskq[:, :], OP.add)
                                ex = sexp.tile([128, SQ], FP16, tag="ex",
                                               name=f"ex{hh}_{i}")
                                nc.scalar.activation(
                                    ex[:, 0:ncol], scw[:, 0:ncol],
                                    AF.Exp, scale=0.125)
                                nc.tensor.matmul(
                                    zp[:, off:SQ],
                                    vaS[i][:, 65*hh:65*(hh+1)],
                                    ex[:, 0:ncol],
                                    start=(i == 0), stop=(i == nsk - 1))
                            nc.scalar.activation(zT[t][lo:lo+64, :],
                                                 zp[0:64, :], AF.Copy)
                            nc.vector.tensor_copy(
                                stg[0:1, SQ*hh:SQ*(hh+1)], zp[64:65, :])
                            nc.sync.dma_start(
                                den[hh:hh+1, :],
                                stg[0:1, SQ*hh:SQ*(hh+1)])
                return zT

            def attention_finalize(zTp, zT, den):
                """zTn = zT * broadcast(1/den) — one batched reciprocal."""
                zTn = [zTp.tile([128, SQ], FP16, tag=f"zTn{t}",
                                name=f"zTn{t}") for t in range(NT)]
                rcp = zTp.tile([H, SQ], F32R, tag="rcp", name="rcp")
                nc.vector.reciprocal(rcp[:], den[:])
                with tc.tile_pool(name="bcps", bufs=2,
                                  space="PSUM") as bcps:
                    for t in range(NT):
                        bc = bcps.tile([128, SQ], F32, tag="bc",
                                       name=f"bc{t}")
                        nc.tensor.matmul(bc[:], sel[:, 128*t:128*(t+1)],
                                         rcp[:], start=True, stop=True)
                        nc.vector.tensor_tensor(zTn[t][:], zT[t][:],
                                                bc[:], OP.mult)
                return zTn

            # ------- LayerNorm tail: stats rows -> broadcast -> apply -------
            def ln_apply(pool, lntmp, bpool, sum_ps, sq_ps, pre, ln_idx,
                         outs, out_dma=False):
                cg = [C_G1, C_G2, C_G3][ln_idx]
                cbe = [C_BE1, C_BE2, C_BE3][ln_idx]
                mean_r = pool.tile([1, SQ], F32R, tag="mean_r",
                                   name="mean_r", bufs=1)
                nc.vector.tensor_scalar_mul(mean_r[:], sum_ps[:], 1.0 / D)
                msq = pool.tile([1, SQ], F32, tag="lnscr", name="msq",
                                bufs=2)
                nc.vector.tensor_tensor(msq[:], mean_r[:].bitcast(F32),
                                        mean_r[:].bitcast(F32), OP.mult)
                var = pool.tile([1, SQ], F32, tag="lnscr", name="var",
                                bufs=2)
                nc.vector.tensor_scalar_mul(var[:], sq_ps[:], 1.0 / D)
                nc.vector.tensor_tensor(var[:], var[:], msq[:], OP.subtract)
                nc.vector.tensor_scalar_add(var[:], var[:], EPS)
                vrec = pool.tile([1, SQ], F32, tag="lnscr", name="vrec",
                                 bufs=2)
                nc.vector.reciprocal(vrec[:], var[:])
                rstd = pool.tile([1, SQ], F32R, tag="rstd", name="rstd",
                                 bufs=1)
                nc.scalar.activation(rstd[:], vrec[:], AF.Sqrt)
                mb = bpool.tile([128, SQ], F32, tag="mb", name="mb_ps")
                nc.tensor.matmul(mb[:], onesr[0:1, 0:128], mean_r[:],
                                 start=True, stop=True)
                rb = bpool.tile([128, SQ], F32, tag="rb", name="rb_ps")
                nc.tensor.matmul(rb[:], onesr[0:1, 0:128], rstd[:],
                                 start=True, stop=True)
                for t in range(NT):
                    tmp = lntmp.tile([128, SQ], FP16, tag="lt1",
                                     name=f"lt1_{t}")
                    nc.vector.tensor_tensor(tmp[:], pre[t][:],
                                            mb[:], OP.subtract)
                    tmp2 = lntmp.tile([128, SQ], FP16, tag="lt2",
                                      name=f"lt2_{t}")
                    nc.vector.tensor_tensor(tmp2[:], tmp[:], rb[:], OP.mult)
                    if out_dma:
                        o = lntmp.tile([128, SQ], F32, tag="lno",
                                       name=f"lno{t}")
                        nc.vector.tensor_scalar(o[:], tmp2[:],
                                                cols[t][:, cg:cg+1],
                                                cols[t][:, cbe:cbe+1],
                                                OP.mult, OP.add)
                        nc.sync.dma_start(out_d[128*t:128*(t+1), :], o[:])
                    else:
                        nc.vector.tensor_scalar(outs[t][:], tmp2[:],
                                                cols[t][:, cg:cg+1],
                                                cols[t][:, cbe:cbe+1],
                                                OP.mult, OP.add)

            # --------- Wo projection + bias + residual + LayerNorm ----------
            def wo_residual_ln(zT, res, ln_idx, outs):
                with tc.tile_pool(name="wow", bufs=3) as wow, \
                     tc.tile_pool(name="wopre", bufs=1) as wopre, \
                     tc.tile_pool(name="wops", bufs=2, space="PSUM") as wops, \
                     tc.tile_pool(name="lnps", bufs=1, space="PSUM") as lnps, \
                     tc.tile_pool(name="lntmp", bufs=2) as lntmp:
                    sum_ps = lnps.tile([1, SQ], F32, tag="sum", name="sum_ps")
                    sq_ps = lnps.tile([1, SQ], F32, tag="sq", name="sq_ps")
                    pre = [wopre.tile([128, SQ], FP16, tag=f"pre{t}",
                                      name=f"pre{t}") for t in range(NT)]
                    for t in range(NT):
                        slab = wow.tile([128, NT * 128], FP16, tag="woslab",
                                        name=f"wos{t}")
                        nc.sync.dma_start(slab[:, :], wo_d[t, :, :])
                        ps = wops.tile([128, SQ], F32, tag="wops",
                                       name=f"wops{t}")
                        for z in range(NT):
                            nc.tensor.matmul(ps[:], slab[:, 128*z:128*(z+1)],
                                             zT[z][:], start=(z == 0),
                                             stop=(z == NT - 1))
                        # pre = (ps + bo_col) + residual
                        nc.vector.scalar_tensor_tensor(
                            pre[t][:], ps[:], cols[t][:, C_BO:C_BO+1],
                            res[t][:], OP.add, OP.add)
                        xsq = lntmp.tile([128, SQ], FP16, tag="xsq",
                                         name=f"xsq{t}")
                        nc.scalar.activation(xsq[:], pre[t][:], AF.Square)
                        nc.tensor.matmul(sum_ps[:], onescol[:, 0:1],
                                         pre[t][:], start=(t == 0),
                                         stop=(t == NT - 1))
                        nc.tensor.matmul(sq_ps[:], onescol[:, 0:1], xsq[:],
                                         start=(t == 0), stop=(t == NT - 1))
                    ln_apply(wopre, lntmp, lnps, sum_ps, sq_ps, pre,
                             ln_idx, outs)

            # ====================== self-attention ==========================
            with tc.tile_pool(name="zTp_s", bufs=1) as zTp_s:
                xqT = [zTp_s.tile([128, SQ], FP16, tag=f"xqT{t}",
                                  name=f"xqT{t}") for t in range(NT)]
                for t in range(NT):
                    nc.sync.dma_start(xqT[t][:], xqT_d[128*t:128*(t+1), :])
                den_s = zTp_s.tile([H, SQ], F32, tag="den_s", name="den_s")
                den_x = zTp_s.tile([H, SQ], F32, tag="den_x", name="den_x")
                with tc.tile_pool(name="attp_s", bufs=1) as attp:
                    maskq = attp.tile([128, 64], FP16, tag="maskq",
                                      name="maskq")
                    nc.sync.dma_start(maskq[:, :], maskq_d[:, :])
                    xT = [attp.tile([128, S], FP16, tag=f"xT{t}",
                                    name=f"xT{t}") for t in range(NT)]
                    for t in range(NT):
                        nc.sync.dma_start(xT[t][:], xT_d[128*t:128*(t+1), :])
                    vaS_s = attention_v(attp, xT, wv_s_d, bv_s_d)
                    load_small_tensors()
                    zT_s = attention(attp, zTp_s, xT, wq_s_d, wk_s_d,
                                     vaS_s, C_BQS, C_BKS, xqT, maskq,
                                     den_s)

                # ------------- cross-attention (V prefetched) ---------------
                with tc.tile_pool(name="attp_x", bufs=1) as attp_x:
                    encT = [attp_x.tile([128, S], FP16, tag=f"encT{t}",
                                        name=f"encT{t}")
                            for t in range(NT)]
                    for t in range(NT):
                        nc.sync.dma_start(encT[t][:],
                                          encT_d[128*t:128*(t+1), :])
                    # cross V depends only on enc -> emitted before the
                    # self finalize/Wo/LN so it fills PE during those tails
                    vaS_x = attention_v(attp_x, encT, wv_x_d, bv_x_d)
                    zTn_s = attention_finalize(zTp_s, zT_s, den_s)
                    wo_residual_ln(zTn_s, xqT, 0, x1T)
                    # cross zT reuses the self zT slots (same tags)
                    zT_x = attention(attp_x, zTp_s, encT, wq_x_d,
                                     wk_x_d, vaS_x, C_BQX, C_BKX,
                                     x1T, None, den_x)
                    zTn_x = attention_finalize(zTp_s, zT_x, den_x)
                wo_residual_ln(zTn_x, x1T, 1, x2T)

            # ============================ FFN ===============================
            with tc.tile_pool(name="ffnp", bufs=1) as ffnp:
                b1c = ffnp.tile([128, NF], F32, tag="b1c", name="b1c")
                nc.sync.dma_start(b1c[:, :], b1c_d[:, :])
                hT = [ffnp.tile([128, SQ], FP16, tag=f"hT{f}",
                                name=f"hT{f}") for f in range(NF)]
                w2p_cm = tc.tile_pool(name="w2p", bufs=2)
                w2p = w2p_cm.__enter__()
                w2_slab0 = None
                with tc.tile_pool(name="w1p", bufs=3) as w1p, \
                     tc.tile_pool(name="hps", bufs=2, space="PSUM") as hps:
                    for f in range(NF):
                        slab = w1p.tile([128, NT * 128], FP16, tag="w1slab",
                                        name=f"w1s{f}")
                        nc.sync.dma_start(slab[:, :], w1_d[f, :, :])
                        ps = hps.tile([128, SQ], F32, tag="hp", name=f"hp{f}")
                        for d in range(NT):
                            nc.tensor.matmul(ps[:], slab[:, 128*d:128*(d+1)],
                                             x2T[d][:], start=(d == 0),
                                             stop=(d == NT - 1))
                        # h = relu(ps + b1) on the (otherwise idle) ACT engine
                        nc.scalar.activation(hT[f][:], ps[:], AF.Relu,
                                             bias=b1c[:, f:f+1])
                        if f == NF - 2:
                            # prefetch the first two W2 slabs
                            w2_slab0 = [
                                w2p.tile([128, NF * 128], FP16,
                                         tag="w2slab", name=f"w2s{t}")
                                for t in range(2)]
                            for t in range(2):
                                nc.sync.dma_start(w2_slab0[t][:, :],
                                                  w2_d[t, :, :])
                with tc.tile_pool(name="ops", bufs=2, space="PSUM") as ops, \
                     tc.tile_pool(name="l3ps", bufs=1, space="PSUM") as l3ps, \
                     tc.tile_pool(name="l3tmp", bufs=2) as l3tmp:
                    sum_ps = l3ps.tile([1, SQ], F32, tag="sum3", name="sum3")
                    sq_ps = l3ps.tile([1, SQ], F32, tag="sq3", name="sq3")
                    pre = [ffnp.tile([128, SQ], FP16, tag=f"opre{t}",
                                     name=f"opre{t}") for t in range(NT)]
                    for t in range(NT):
                        if t < 2:
                            slab = w2_slab0[t]
                        else:
                            slab = w2p.tile([128, NF * 128], FP16,
                                            tag="w2slab", name=f"w2s{t}")
                            nc.sync.dma_start(slab[:, :], w2_d[t, :, :])
                        ps = ops.tile([128, SQ], F32, tag="op", name=f"op{t}")
                        for f in range(NF):
                            nc.tensor.matmul(ps[:], slab[:, 128*f:128*(f+1)],
                                             hT[f][:], start=(f == 0),
                                             stop=(f == NF - 1))
                        nc.vector.scalar_tensor_tensor(
                            pre[t][:], ps[:], cols[t][:, C_B2:C_B2+1],
                            x2T[t][:], OP.add, OP.add)
                        xsq = l3tmp.tile([128, SQ], FP16, tag="xsq3",
                                         name=f"xsq3{t}")
                        nc.scalar.activation(xsq[:], pre[t][:], AF.Square)
                        nc.tensor.matmul(sum_ps[:], onescol[:, 0:1],
                                         pre[t][:], start=(t == 0),
                                         stop=(t == NT - 1))
                        nc.tensor.matmul(sq_ps[:], onescol[:, 0:1], xsq[:],
                                         start=(t == 0), stop=(t == NT - 1))
                    ln_apply(ffnp, l3tmp, l3ps, sum_ps, sq_ps, pre, 2,
                             None, out_dma=True)
                w2p_cm.__exit__(None, None, None)

        if loop_k == 1:
            body()
        else:
            with tc.For_i(0, loop_k, 1):
                body()
    nc.compile()
    return nc


# ======================= host-side wrapper ==================================

_RUNNER_CACHE = {}


class _SpmdRunner:
    """Compile nc once, run on 8 axon cores via PJRT shard_map."""

    def __init__(self, nc, n_cores=8):
        import jax
        from jax.sharding import Mesh, PartitionSpec
        from jax.experimental.shard_map import shard_map
        from concourse import bass2jax
        from concourse.bass2jax import _bass_exec_p, install_neuronx_cc_hook
        install_neuronx_cc_hook()
        self.jax = jax
        self.n_cores = n_cores
        partition_name = (nc.partition_id_tensor.name
                          if nc.partition_id_tensor else None)
        in_names, out_names, out_avals, zero_outs = [], [], [], []
        for alloc in nc.m.functions[0].allocations:
            if not isinstance(alloc, mybir.MemoryLocationSet):
                continue
            name = alloc.memorylocations[0].name
            if alloc.kind == "ExternalInput":
                if name != partition_name:
                    in_names.append(name)
            elif alloc.kind == "ExternalOutput":
                out_names.append(name)
                shape = tuple(alloc.tensor_shape)
                dtype = mybir.dt.np(alloc.dtype)
                out_avals.append(jax.core.ShapedArray(shape, dtype))
                zero_outs.append(np.zeros(shape, dtype))
        self.in_names = in_names
        self.out_names = out_names
        self.out_avals = out_avals
        self.zero_outs = zero_outs
        n_params = len(in_names)
        n_outs = len(out_avals)
        all_in_names = in_names + out_names
        if partition_name is not None:
            all_in_names.append(partition_name)

        def _body(*args):
            operands = list(args)
            if partition_name is not None:
                operands.append(bass2jax.partition_id_tensor())
            outs = _bass_exec_p.bind(
                *operands,
                out_avals=tuple(out_avals),
                in_names=tuple(all_in_names),
                out_names=tuple(out_names),
                lowering_input_output_aliases=(),
                sim_require_finite=True,
                sim_require_nnan=True,
                nc=nc,
            )
            return tuple(outs)

        devices = jax.devices()[:n_cores]
        self.mesh = Mesh(np.asarray(devices), ("core",))
        in_specs = (PartitionSpec("core"),) * (n_params + n_outs)
        out_specs = (PartitionSpec("core"),) * n_outs
        self.fn = jax.jit(
            shard_map(_body, mesh=self.mesh, in_specs=in_specs,
                      out_specs=out_specs, check_rep=False),
            keep_unused=True)
        self.n_params = n_params
        self.PartitionSpec = PartitionSpec

    def prepare(self, in_maps):
        per_core = [[np.asarray(m[name]) for name in self.in_names]
                    for m in in_maps]
        concat_in = [
            np.concatenate([per_core[c][i] for c in range(self.n_cores)], 0)
            for i in range(self.n_params)]
        concat_zeros = [
            np.zeros((self.n_cores * z.shape[0], *z.shape[1:]), z.dtype)
            for z in self.zero_outs]
        sharding = self.jax.sharding.NamedSharding(
            self.mesh, self.PartitionSpec("core"))
        self.dev_args = [self.jax.device_put(a, sharding)
                         for a in (*concat_in, *concat_zeros)]

    def run(self):
        outs = self.fn(*self.dev_args)
        self.jax.block_until_ready(outs)
        return outs

    def results(self, outs):
        res = []
        for c in range(self.n_cores):
            d = {}
            for i, name in enumerate(self.out_names):
                d[name] = np.asarray(outs[i]).reshape(
                    self.n_cores, *self.out_avals[i].shape)[c]
            res.append(d)
        return res


def _stack_w(w):  # [H, D, DK] -> [D, H*DK]
    return np.ascontiguousarray(
        np.transpose(np.asarray(w, np.float32), (1, 0, 2)).reshape(D, H * DK))


def _tile_lhs(w):
    """[Din, Dout] -> [Dout//128 slabs, 128, (Din//128)*128]: slab t has
    columns 128t:128(t+1), laid out [p, d*128 + c] with
    slab[t][p, 128d + c] = w[128d + p, 128t + c]."""
    w = np.asarray(w, np.float32)
    din, dout = w.shape
    a = w.reshape(din // 128, 128, dout // 128, 128)       # [d, p, t, c]
    return np.ascontiguousarray(a.transpose(2, 1, 0, 3).reshape(
        dout // 128, 128, (din // 128) * 128))


def _tile_rhs(w):
    """[Din, Dout] -> [2*(Din//128) slabs, 128, 512]: slab s*(Din//128)+d =
    w[128d:128(d+1), 512s:512(s+1)] (for the V projection rhs)."""
    w = np.asarray(w, np.float32)
    din, dout = w.shape
    a = w.reshape(din // 128, 128, dout // 512, 512)       # [d, p, s, c]
    return np.ascontiguousarray(a.transpose(2, 0, 1, 3).reshape(
        (dout // 512) * (din // 128), 128, 512))


def _row(b):  # [H, DK] or [N] -> [1, N]
    return np.ascontiguousarray(np.asarray(b, np.float32).reshape(1, -1))


def _qidx(h):
    """Interleaved 64-query blocks for core-half h: blocks {2j+h}."""
    return np.concatenate(
        [64 * (2 * j + h) + np.arange(64) for j in range(8)])


def make_in_maps(x, enc, mask, Wq_self, bq_self, Wk_self, bk_self, Wv_self,
                 bv_self, Wq_x, bq_x, Wk_x, bk_x, Wv_x, bv_x, Wo, bo,
                 W1, b1, W2, b2, g1, be1, g2, be2, g3, be3):
    import ml_dtypes
    f32 = np.float32
    bf = np.float16
    x = np.asarray(x, f32)
    enc = np.asarray(enc, f32)
    wq_s = _tile_lhs(_stack_w(Wq_self)).astype(bf)
    wk_s = _tile_lhs(_stack_w(Wk_self)).astype(bf)
    wv_s = _tile_rhs(_stack_w(Wv_self)).astype(bf)
    wq_x = _tile_lhs(_stack_w(Wq_x)).astype(bf)
    wk_x = _tile_lhs(_stack_w(Wk_x)).astype(bf)
    wv_x = _tile_rhs(_stack_w(Wv_x)).astype(bf)
    cols = np.stack([np.asarray(a, f32).reshape(D) for a in
                     (g1, be1, g2, be2, g3, be3,
                      np.asarray(bq_self, f32).reshape(D),
                      np.asarray(bk_self, f32).reshape(D),
                      np.asarray(bq_x, f32).reshape(D),
                      np.asarray(bk_x, f32).reshape(D),
                      bo, b2)], axis=1)
    cols = np.ascontiguousarray(cols)
    sel = np.zeros((H, NT * 128), f32)
    for t in range(NT):
        sel[2 * t, 128 * t: 128 * t + 64] = 1.0
        sel[2 * t + 1, 128 * t + 64: 128 * (t + 1)] = 1.0
    common = {
        "wq_s": wq_s, "wk_s": wk_s, "wv_s": wv_s,
        "wq_x": wq_x, "wk_x": wk_x, "wv_x": wv_x,
        "bv_s": _row(bv_self), "bv_x": _row(bv_x),
        "wo": _tile_lhs(np.asarray(Wo, f32)).astype(bf),
        "w1": _tile_lhs(np.asarray(W1, f32)).astype(bf),
        "w2": _tile_lhs(np.asarray(W2, f32)).astype(bf),
        "b1c": np.ascontiguousarray(
            np.asarray(b1, f32).reshape(NF, 128).T),
        "cols": cols,
        "sel": sel,
        "onesr": np.ones((1, SQ), f32),
        "onescol": np.ones((128, 1), bf),
        "vones": np.ones((128, H), bf),
    }
    in_maps = []
    for c in range(8):
        b = c // 2
        h = c % 2
        qidx = _qidx(h)
        xTb = np.ascontiguousarray(x[b].T)
        # diagonal-block mask: key k vs query 64h+q within the block
        kk = np.arange(128)[:, None]
        qq = np.arange(64)[None, :]
        m = np.where(kk > 64 * h + qq, MASK_NEG, 0.0).astype(f32)
        in_maps.append({
            "xT": xTb.astype(bf),
            "xqT": np.ascontiguousarray(xTb[:, qidx]).astype(bf),
            "encT": np.ascontiguousarray(enc[b].T).astype(bf),
            "maskq": m.astype(bf),
            **common,
        })
    return in_maps


def get_runner(loop_k=1):
    if loop_k not in _RUNNER_CACHE:
        nc = build_decoder(loop_k=loop_k)
        _RUNNER_CACHE[loop_k] = _SpmdRunner(nc, 8)
    return _RUNNER_CACHE[loop_k]


def kernel(**inputs):
    in_maps = make_in_maps(**inputs)
    r = get_runner()
    r.prepare(in_maps)
    outs = r.run()
    res = r.results(outs)
    out = np.empty((B, S, D), np.float32)
    for c in range(8):
        b, h = c // 2, c % 2
        out[b, _qidx(h), :] = res[c]["out"].T
    return out
